# revision 28
# baseline (speedup 1.0000x reference)
"""Multi-head attention (B=4, S=2048, D=512, H=8, DH=64) on 8 TRN2 NeuronCores.

Sharding: core c handles batch b = c//2 and head-group g = c%2 (4 of the 8
heads).  Each core computes its QKV projection (columns of W_qkv for its
heads), attention for its 4 heads, and a partial output projection
(rows of W_out for its heads).  The host sums the two partials per batch
and adds the bias.

Per-core device layout (compute in bf16, fp32 PSUM accumulation):
  - host passes x[b] pre-transposed (xT [D, S]); the QKV projection then
    needs no on-device transpose.
  - q/k live PACKED two heads per 128-partition chunk (head 2j in
    partitions 0:64, head 2j+1 in 64:128).  Score matmuls contract over
    just the 64 real rows and use TensorE row tiling (tile_position
    (0,0) / (64,0)) so both heads of a pair compute CONCURRENTLY in the
    two halves of the PE array.
  - exp(scale*s) is fused into the PSUM->SBUF drain on the Scalar
    engine, one ACTIVATE per (pair, k-block) covering both heads
    (128x1024) to amortize the ~350-cycle ACT fixed overhead.  ACT is
    the pacing engine in steady state; everything else hides under it.
  - attn @ V uses V in natural [token, dh] layout augmented with a ones
    column: one PSUM accumulation produces outT_unnorm [dh, q] AND the
    softmax denominator row.
  - normalization: reciprocal_approx_fast of the denominator row,
    DMA-shift to partition 0, gpsimd broadcast across 64 partitions,
    multiply on the Vector engine.
  - output projection: lhsT = outT [128 (2 heads), 128 q] against the
    matching W_out rows, accumulated over head pairs; the result is
    DMA'd to DRAM straight out of PSUM.
  - phase A (QKV projection) runs k-projection-first so the first exp
    lands on ACT ~8us into the kernel; the remaining projection blocks
    are woven into phase B's ACT-bound groups as TensorE filler, using
    a dedicated 1-bank PSUM ring so they never block the score pipeline.
"""

import sys

for _p in ("/opt/trn_rl_repo", "/root/.axon_site/_ro/trn_rl_repo"):
    if _p not in sys.path:
        sys.path.append(_p)

from collections import deque

import ml_dtypes
import numpy as np

import concourse.bass as bass
import concourse.tile as tile
from concourse import bacc, mybir

F32 = mybir.dt.float32
BF16 = mybir.dt.bfloat16
AF = mybir.ActivationFunctionType

# Problem dims (hardcoded per the grading contract).
B, S, D = 4, 2048, 512
H, DH = 8, 64
INNER = H * DH
HL = 4                # heads per core
DO = D                # output dim
QT = 512              # query tile
SCALE = DH ** -0.5

N_CORES = 8


def build_nc():
    KB = S // 128         # k-token blocks
    DC = D // 128         # contraction chunks for the projections
    QKF = 2 * HL * DH     # q+k feature count per core
    MQK = QKF // 128      # qk feature blocks (2 heads each): m0,m1=q m2,m3=k
    VF = HL * DH          # v feature count per core
    NQT = S // QT         # query tiles
    NP = HL // 2          # head pairs

    CDT = BF16

    nc = bacc.Bacc(
        "TRN2", target_bir_lowering=False, debug=False, num_devices=N_CORES
    )
    # All inputs arrive pre-arranged in SBUF-tile-major layouts so each DMA
    # line is 4-16KB contiguous per partition (descriptor-rate bound
    # otherwise).
    xT = nc.dram_tensor("xT", [128, 4, DC, 512], BF16, kind="ExternalInput").ap()
    wqk = nc.dram_tensor(
        "wqk", [128, MQK, DC, 128], BF16, kind="ExternalInput"
    ).ap()
    wv = nc.dram_tensor("wv", [128, DC, VF], BF16, kind="ExternalInput").ap()
    wo = nc.dram_tensor("wo", [128, NP, DO], BF16, kind="ExternalInput").ap()
    y = nc.dram_tensor("y", [S, DO], F32, kind="ExternalOutput").ap()

    with tile.TileContext(nc) as tc:
        with (
            tc.tile_pool(name="weights", bufs=1) as wpool,
            tc.tile_pool(name="big", bufs=1) as big,
        ):
            # ---- loads, ordered by first use: k-projection weights and
            # the first token half lead so the PE starts ~3us in.
            wqk_sb = wpool.tile([128, MQK, DC, 128], CDT)

            def load_wqk(m):
                nc.sync.dma_start(out=wqk_sb[:, m], in_=wqk[:, m])

            xT_sb = big.tile([128, 4, DC, 512], CDT)

            def load_x(t, split=1):
                for h in range(split):
                    c0, c1 = h * DC // split, (h + 1) * DC // split
                    nc.sync.dma_start(
                        out=xT_sb[:, t, c0:c1], in_=xT[:, t, c0:c1]
                    )

            wv_sb = wpool.tile([128, DC, VF], CDT)
            wo_sb = wpool.tile([128, NP, DO], CDT)
            load_wqk(2)
            load_x(0, split=2)
            load_wqk(0)
            nc.sync.dma_start(out=wv_sb, in_=wv)
            load_x(1)
            load_x(2)
            load_x(3)
            load_wqk(3)
            load_wqk(1)
            nc.sync.dma_start(out=wo_sb, in_=wo)

            # qkT zero-padded: chunk h = qT of head h (real rows
            # (h%2)*64..+64, rest 0), chunk HL+h = kT of head h (same
            # padding); score matmuls contract over K=128 (the zero rows
            # contribute nothing -- sub-128 contraction is not supported
            # by this HW stack, it crashes the runtime).
            # Warm-up while the first DMAs land: a 1-element exp pulls the
            # ACT table load (~2.7us) off the critical path, and a dozen
            # junk matmuls ramp the PE out of its low p-state (the clock
            # needs ~3us of continuous work to reach 2.4GHz).
            scratch = big.tile([128, 512], CDT)
            nc.gpsimd.memset(scratch, 0.5)
            scr_exp = big.tile([1, 1], CDT)
            nc.scalar.activation(
                out=scr_exp, in_=scratch[0:1, 0:1], func=AF.Exp
            )

            qkT = big.tile([128, 2 * HL, S], CDT)
            # zero the pad halves on the (otherwise idle) gpsimd engine,
            # chunk by chunk in first-use order so the prefix copies don't
            # wait on one monolithic 14us memset.
            for ch in (4, 5, 0, 1, 6, 7, 2, 3):
                nc.gpsimd.memset(qkT[:, ch, :], 0.0)
            vaug = big.tile([128, KB, HL, DH + 1], CDT)
            ones_col = vaug[:, :, :, DH:DH + 1]
            nc.vector.memset(ones_col, 1.0)

            def qk_mms(ps, m, t):
                for c in range(DC):
                    nc.tensor.matmul(
                        ps,
                        lhsT=wqk_sb[:, m, c, :],
                        rhs=xT_sb[:, t, c, :],
                        start=(c == 0),
                        stop=(c == DC - 1),
                    )

            def v_mms(ps, tb):
                t, sub = divmod(tb, 4)
                for c in range(DC):
                    nc.tensor.matmul(
                        ps,
                        lhsT=xT_sb[:, t, c, sub * 128:(sub + 1) * 128],
                        rhs=wv_sb[:, c, :],
                        start=(c == 0),
                        stop=(c == DC - 1),
                    )

            # Blocking prefix (pipelined through a 4-bank ring that is
            # released before the phase-B pools open): k pair 0 over all
            # tokens, v token-blocks 0..3, q pair 0 tile 0.
            with tc.tile_pool(name="psApre", bufs=4, space="PSUM") as psApre:
                wups = psApre.tile([128, 512], F32, tag="pre", name="wup")
                for _ in range(12):
                    nc.tensor.matmul(
                        wups,
                        lhsT=scratch[:, 0:128],
                        rhs=scratch,
                        skip_group_check=True,
                    )
                def qk_unit_pre(m, t):
                    ps = psApre.tile([128, 512], F32, tag="pre", name="psqk")
                    qk_mms(ps, m, t)
                    base = HL if m >= 2 else 0
                    hp = 2 * (m % 2)
                    sl = slice(t * 512, (t + 1) * 512)
                    nc.scalar.copy(
                        out=qkT[0:64, base + hp, sl], in_=ps[0:64, :]
                    )
                    nc.vector.tensor_copy(
                        out=qkT[64:128, base + hp + 1, sl], in_=ps[64:128, :]
                    )

                def v_unit_pre(t):
                    ps = psApre.tile([128, 512], F32, tag="pre", name="psv")
                    v_mms(ps[:, 0:VF], t)
                    nc.scalar.copy(
                        out=vaug[:, t, :, 0:DH],
                        in_=ps[:, 0:VF].rearrange("p (h e) -> p h e", h=HL),
                    )

                qk_unit_pre(2, 0)
                qk_unit_pre(0, 0)
                for t in range(5):
                    v_unit_pre(t)

            # PSUM budget (8 banks): psA 1 (filler ring) + psS 4 (score
            # double-buffer) + psB2 3 (AV accumulators + proj).
            with (
                tc.tile_pool(name="psA", bufs=1, space="PSUM") as psA,
                tc.tile_pool(name="psS", bufs=2, space="PSUM") as psS,
                tc.tile_pool(name="psB2", bufs=3, space="PSUM") as psB2,
                tc.tile_pool(name="attnp", bufs=2) as attnp,
                tc.tile_pool(name="outp", bufs=2) as outp,
                tc.tile_pool(name="smalls", bufs=3) as smalls,
            ):
                # ---- leftover phase A as fillers (DVE copies) ----
                def qk_unit(m, t):
                    ps = psA.tile([128, 512], F32, tag="fa", name="psqk")
                    qk_mms(ps, m, t)
                    base = HL if m >= 2 else 0
                    hp = 2 * (m % 2)
                    sl = slice(t * 512, (t + 1) * 512)
                    nc.vector.tensor_copy(
                        out=qkT[0:64, base + hp, sl], in_=ps[0:64, :]
                    )
                    nc.vector.tensor_copy(
                        out=qkT[64:128, base + hp + 1, sl], in_=ps[64:128, :]
                    )

                def v_unit(t):
                    ps = psA.tile([128, 512], F32, tag="fa", name="psv")
                    v_mms(ps[:, 0:VF], t)
                    nc.vector.tensor_copy(
                        out=vaug[:, t, :, 0:DH],
                        in_=ps[:, 0:VF].rearrange("p (h e) -> p h e", h=HL),
                    )

                pending = deque()

                def F(fn, *a):
                    pending.append(lambda: fn(*a))

                # Deadline-ordered fillers (tile-0/pair-0 slot g pops
                # filler #g): v block t before AV(t) at group t+1 (position
                # <= t); k tile t before score kb=4t (position <= 4t);
                # pair-1 q/k before pair 1 starts (position <= 15).
                F(qk_unit, 2, 1); F(v_unit, 5)
                F(qk_unit, 2, 2); F(v_unit, 6)
                F(v_unit, 7)
                F(qk_unit, 2, 3)
                for t in range(8, 16):
                    F(v_unit, t)
                F(qk_unit, 3, 0)
                F(qk_unit, 1, 0)
                # Late fillers: spaced one-apart (None = skip a slot) so
                # each filler's PSUM-ring copy completes before the next
                # filler's matmul, even when the DVE is busy with a
                # normalize chain -- otherwise the in-order PE queue stalls
                # at every pair boundary.
                SK = None
                extras = {
                    0: [(3, 1), SK, (3, 2), SK, (3, 3), SK, (0, 1), SK,
                        (1, 1), SK, (0, 2), SK, (1, 2), SK, (0, 3), SK,
                        (1, 3)],
                }

                # ---- phase B ----
                def slot(kb=KB):
                    if pending:
                        if getattr(pending[0], "is_proj", False) and kb < 10:
                            return
                        pending.popleft()()

                def drain_avp(avp, avc):
                    """Copy both AV accumulators to SBUF, freeing their
                    PSUM banks for the next pair's accumulation."""
                    for i in range(2):
                        avc[i] = smalls.tile(
                            [DH + 1, QT], F32, tag="avc", name="avc"
                        )
                        nc.vector.tensor_copy(out=avc[i], in_=avp[i])

                def normalize(h, avc, outT):
                    j, i = divmod(h, 2)
                    a = avc[i]
                    # shift the denominator row to partition 0 first:
                    # reciprocal_approx_fast mis-executes on HW when its
                    # APs have a nonzero base partition.
                    rd0 = smalls.tile([1, QT], F32, tag="rd0")
                    nc.sync.dma_start(out=rd0, in_=a[DH:DH + 1, :])
                    rr0 = smalls.tile([1, QT], F32, tag="rr0")
                    nc.vector.reciprocal_approx_fast(out=rr0, in_=rd0)
                    rb = smalls.tile([64, QT], F32, tag="rb")
                    nc.gpsimd.partition_broadcast(rb, rr0, channels=64)
                    if i == 0:
                        nc.gpsimd.tensor_mul(
                            outT[0:64, j, :], a[0:DH, :], rb
                        )
                    else:
                        ot = smalls.tile([64, QT], CDT, tag="ot")
                        nc.gpsimd.tensor_mul(ot, a[0:DH, :], rb)
                        nc.sync.dma_start(out=outT[64:128, j, :], in_=ot)

                def proj_unit(outT, n, qb):
                    yps = psB2.tile([128, DO], F32, tag="bank", name="yps")

                    for c in range(NP):
                        nc.tensor.matmul(
                            yps,
                            lhsT=outT[:, c, qb * 128:(qb + 1) * 128],
                            rhs=wo_sb[:, c, :],
                            start=(c == 0),
                            stop=(c == NP - 1),
                            skip_group_check=True,
                        )
                    ysb = smalls.tile([128, DO], F32, tag="ysb", name="ysb")
                    nc.vector.tensor_copy(out=ysb, in_=yps)
                    nc.sync.dma_start(
                        out=y[n * QT + qb * 128:n * QT + (qb + 1) * 128, :],
                        in_=ysb,
                    )

                for n in range(NQT):
                    outT = outp.tile([128, NP, QT], CDT, tag="outT")
                    nsl = slice(n * QT, (n + 1) * QT)
                    for j in range(NP):
                        atp = attnp.tile([128, KB, 2, QT], CDT, tag="atp",
                                         name="atp")
                        avp = [None, None]
                        avcs = [None, None]

                        def av_pair(kb, j=j, atp=atp, avp=avp):
                            for i in range(2):
                                if kb == 0:
                                    avp[i] = psB2.tile(
                                        [DH + 1, QT], F32, tag="bank",
                                        name="avp",
                                    )
                                nc.tensor.matmul(
                                    avp[i],
                                    lhsT=vaug[:, kb, 2 * j + i, :],
                                    rhs=atp[:, kb, i, :],
                                    start=(kb == 0),
                                    stop=(kb == KB - 1),
                                    skip_group_check=True,
                                )

                        def av_tail(avp=avp, avcs=avcs, av_pair=av_pair):
                            """Last AV chunk + drain of both accumulators,
                            popped FIRST in the next pair so its banks free
                            before the next pair's accumulators allocate."""
                            av_pair(KB - 1)
                            drain_avp(avp, avcs)

                        for kb in range(KB):
                            ksl = slice(kb * 128, (kb + 1) * 128)
                            ps = psS.tile([128, 2, 512], F32, tag="sc")
                            for i in range(2):
                                h = 2 * j + i
                                nc.tensor.matmul(
                                    ps[:, i, :],
                                    lhsT=qkT[:, HL + h, ksl],
                                    rhs=qkT[:, h, nsl],
                                    skip_group_check=True,
                                )
                            nc.scalar.activation(
                                out=atp[:, kb, :, :], in_=ps,
                                func=AF.Exp, scale=SCALE,
                            )
                            slot(kb)
                            if kb > 0:
                                av_pair(kb - 1)
                        # pair tail: last AV chunk + accumulator drain +
                        # normalizes pop at the START of the next pair's
                        # slots (ahead of leftover fillers).
                        for mt in extras.pop(2 * n + j, []):
                            if mt is None:
                                pending.append(lambda: None)
                            else:
                                F(qk_unit, mt[0], mt[1])
                        pending.appendleft(
                            lambda h=2 * j, a=avcs, o=outT:
                            normalize(h, a, o)
                        )
                        pending.appendleft(
                            lambda h=2 * j + 1, a=avcs, o=outT:
                            normalize(h, a, o)
                        )
                        pending.appendleft(av_tail)
                    for qb in range(QT // 128):
                        pu = (lambda o=outT, n=n, qb=qb:
                              proj_unit(o, n, qb))
                        pu.is_proj = True
                        pending.append(pu)

                drain_i = 0
                while pending:
                    item = pending.popleft()
                    item()
                    drain_i += 1
                    if drain_i <= 4 and not getattr(item, "is_proj", False):
                        for _ in range(3):
                            wu = psA.tile([128, 512], F32, tag="fa",
                                          name="wu")
                            nc.tensor.matmul(
                                wu,
                                lhsT=scratch[:, 0:128],
                                rhs=scratch,
                                skip_group_check=True,
                            )

    nc.compile()
    return nc


def shard_inputs(x, W_qkv, W_out):
    """Full inputs -> list of 8 per-core input maps."""
    dt = ml_dtypes.bfloat16
    in_maps = []
    for c in range(N_CORES):
        b, g = divmod(c, 2)
        qcols = W_qkv[:, g * 256:(g + 1) * 256]
        kcols = W_qkv[:, INNER + g * 256:INNER + (g + 1) * 256]
        vcols = W_qkv[:, 2 * INNER + g * 256:2 * INNER + (g + 1) * 256]
        # device-tile-major layouts (see dram_tensor decls in build_nc)
        xTl = (x[b].T.reshape(4, 128, 4, 512)        # (c, p, t, u)
               .transpose(1, 2, 0, 3))               # (p, t, c, u)
        wqkl = (np.concatenate([qcols, kcols], axis=1)
                .reshape(4, 128, 4, 128)             # (c, p, m, f)
                .transpose(1, 2, 0, 3))              # (p, m, c, f)
        wvl = vcols.reshape(4, 128, 256).transpose(1, 0, 2)
        wol = W_out[g * 256:(g + 1) * 256, :].reshape(2, 128, 512)            .transpose(1, 0, 2)
        in_maps.append({
            "xT": np.ascontiguousarray(xTl).astype(dt),
            "wqk": np.ascontiguousarray(wqkl).astype(dt),
            "wv": np.ascontiguousarray(wvl).astype(dt),
            "wo": np.ascontiguousarray(wol).astype(dt),
        })
    return in_maps


def gather_output(ys, b_out):
    out = np.empty((B, S, DO), np.float32)
    for b in range(B):
        out[b] = ys[2 * b] + ys[2 * b + 1]
        out[b] += b_out
    return out


_NC_CACHE = {}


def _get_nc():
    if "nc" not in _NC_CACHE:
        _NC_CACHE["nc"] = build_nc()
    return _NC_CACHE["nc"]


def kernel(**inputs):
    x = np.asarray(inputs["x"], np.float32)
    W_qkv = np.asarray(inputs["W_qkv"], np.float32)
    W_out = np.asarray(inputs["W_out"], np.float32)
    b_out = np.asarray(inputs["b_out"], np.float32)

    from concourse.bass_utils import run_bass_kernel_spmd

    nc = _get_nc()
    in_maps = shard_inputs(x, W_qkv, W_out)
    res = run_bass_kernel_spmd(nc, in_maps, core_ids=list(range(N_CORES)))
    ys = [r["y"] for r in res.results]
    return gather_output(ys, b_out)


# revision 29
# speedup vs baseline: 1.7013x; 1.7013x over previous
"""Multi-head attention (B=4, S=2048, D=512, H=8, DH=64) on 8 TRN2 NeuronCores.

Sharding: core c handles batch b = c//2 and head-group g = c%2 (4 of the 8
heads).  Each core computes its QKV projection (columns of W_qkv for its
heads), attention for its 4 heads, and a partial output projection
(rows of W_out for its heads).  The host sums the two partials per batch
and adds the bias.

Per-core device layout (compute in bf16, fp32 PSUM accumulation):
  - host passes x[b] pre-transposed (xT [D, S]); the QKV projection then
    needs no on-device transpose.
  - q/k live PACKED two heads per 128-partition chunk (head 2j in
    partitions 0:64, head 2j+1 in 64:128).  Score matmuls contract over
    just the 64 real rows and use TensorE row tiling (tile_position
    (0,0) / (64,0)) so both heads of a pair compute CONCURRENTLY in the
    two halves of the PE array.
  - exp(scale*s) is fused into the PSUM->SBUF drain on the Scalar
    engine, one ACTIVATE per (pair, k-block) covering both heads
    (128x1024) to amortize the ~350-cycle ACT fixed overhead.  ACT is
    the pacing engine in steady state; everything else hides under it.
  - attn @ V uses V in natural [token, dh] layout augmented with a ones
    column: one PSUM accumulation produces outT_unnorm [dh, q] AND the
    softmax denominator row.
  - normalization: reciprocal_approx_fast of the denominator row,
    DMA-shift to partition 0, gpsimd broadcast across 64 partitions,
    multiply on the Vector engine.
  - output projection: lhsT = outT [128 (2 heads), 128 q] against the
    matching W_out rows, accumulated over head pairs; the result is
    DMA'd to DRAM straight out of PSUM.
  - phase A (QKV projection) runs k-projection-first so the first exp
    lands on ACT ~8us into the kernel; the remaining projection blocks
    are woven into phase B's ACT-bound groups as TensorE filler, using
    a dedicated 1-bank PSUM ring so they never block the score pipeline.
"""

import sys

for _p in ("/opt/trn_rl_repo", "/root/.axon_site/_ro/trn_rl_repo"):
    if _p not in sys.path:
        sys.path.append(_p)

from collections import deque

import ml_dtypes
import numpy as np

import concourse.bass as bass
import concourse.tile as tile
from concourse import bacc, mybir

F32 = mybir.dt.float32
BF16 = mybir.dt.bfloat16
AF = mybir.ActivationFunctionType

# Problem dims (hardcoded per the grading contract).
B, S, D = 4, 2048, 512
H, DH = 8, 64
INNER = H * DH
HL = 4                # heads per core
DO = D                # output dim
QT = 512              # query tile
SCALE = DH ** -0.5

N_CORES = 8


def build_nc():
    KB = S // 128         # k-token blocks
    DC = D // 128         # contraction chunks for the projections
    QKF = 2 * HL * DH     # q+k feature count per core
    MQK = QKF // 128      # qk feature blocks (2 heads each): m0,m1=q m2,m3=k
    VF = HL * DH          # v feature count per core
    NQT = S // QT         # query tiles
    NP = HL // 2          # head pairs

    CDT = BF16

    nc = bacc.Bacc(
        "TRN2", target_bir_lowering=False, debug=False, num_devices=N_CORES
    )
    # All inputs arrive pre-arranged in SBUF-tile-major layouts so each DMA
    # line is 4-16KB contiguous per partition (descriptor-rate bound
    # otherwise).
    xT = nc.dram_tensor("xT", [128, 4, DC, 512], BF16, kind="ExternalInput").ap()
    wqk = nc.dram_tensor(
        "wqk", [128, MQK, DC, 128], BF16, kind="ExternalInput"
    ).ap()
    wv = nc.dram_tensor("wv", [128, DC, VF], BF16, kind="ExternalInput").ap()
    wo = nc.dram_tensor("wo", [128, NP, DO], BF16, kind="ExternalInput").ap()
    y = nc.dram_tensor("y", [S, DO], F32, kind="ExternalOutput").ap()

    with tile.TileContext(nc) as tc:
        with (
            tc.tile_pool(name="weights", bufs=1) as wpool,
            tc.tile_pool(name="big", bufs=1) as big,
        ):
            # ---- loads, ordered by first use: k-projection weights and
            # the first token half lead so the PE starts ~3us in.
            wqk_sb = wpool.tile([128, MQK, DC, 128], CDT)

            def load_wqk(m):
                nc.sync.dma_start(out=wqk_sb[:, m], in_=wqk[:, m])

            xT_sb = big.tile([128, 4, DC, 512], CDT)

            def load_x(t, split=1):
                for h in range(split):
                    c0, c1 = h * DC // split, (h + 1) * DC // split
                    nc.sync.dma_start(
                        out=xT_sb[:, t, c0:c1], in_=xT[:, t, c0:c1]
                    )

            wv_sb = wpool.tile([128, DC, VF], CDT)
            wo_sb = wpool.tile([128, NP, DO], CDT)
            load_wqk(2)
            load_x(0, split=2)
            load_wqk(0)
            nc.sync.dma_start(out=wv_sb, in_=wv)
            load_x(1)
            load_x(2)
            load_x(3)
            load_wqk(3)
            load_wqk(1)
            nc.sync.dma_start(out=wo_sb, in_=wo)

            # qkT zero-padded: chunk h = qT of head h (real rows
            # (h%2)*64..+64, rest 0), chunk HL+h = kT of head h (same
            # padding); score matmuls contract over K=128 (the zero rows
            # contribute nothing -- sub-128 contraction is not supported
            # by this HW stack, it crashes the runtime).
            # Warm-up while the first DMAs land: a 1-element exp pulls the
            # ACT table load (~2.7us) off the critical path, and a dozen
            # junk matmuls ramp the PE out of its low p-state (the clock
            # needs ~3us of continuous work to reach 2.4GHz).
            scratch = big.tile([128, 512], CDT)
            nc.gpsimd.memset(scratch, 0.5)
            scr_exp = big.tile([1, 1], CDT)
            nc.scalar.activation(
                out=scr_exp, in_=scratch[0:1, 0:1], func=AF.Exp
            )

            qkT = big.tile([128, 2 * HL, S], CDT)
            # zero the pad halves on the (otherwise idle) gpsimd engine,
            # chunk by chunk in first-use order so the prefix copies don't
            # wait on one monolithic 14us memset.
            for ch in (4, 5, 0, 1, 6, 7, 2, 3):
                nc.gpsimd.memset(qkT[:, ch, :], 0.0)
            vaug = big.tile([128, KB, HL, DH + 1], CDT)
            ones_col = vaug[:, :, :, DH:DH + 1]
            nc.vector.memset(ones_col, 1.0)

            def qk_mms(ps, m, t):
                for c in range(DC):
                    nc.tensor.matmul(
                        ps,
                        lhsT=wqk_sb[:, m, c, :],
                        rhs=xT_sb[:, t, c, :],
                        start=(c == 0),
                        stop=(c == DC - 1),
                    )

            def v_mms(ps, tb):
                t, sub = divmod(tb, 4)
                for c in range(DC):
                    nc.tensor.matmul(
                        ps,
                        lhsT=xT_sb[:, t, c, sub * 128:(sub + 1) * 128],
                        rhs=wv_sb[:, c, :],
                        start=(c == 0),
                        stop=(c == DC - 1),
                    )

            # Blocking prefix (pipelined through a 4-bank ring that is
            # released before the phase-B pools open): k pair 0 over all
            # tokens, v token-blocks 0..3, q pair 0 tile 0.
            with tc.tile_pool(name="psApre", bufs=4, space="PSUM") as psApre:
                wups = psApre.tile([128, 512], F32, tag="pre", name="wup")
                for _ in range(12):
                    nc.tensor.matmul(
                        wups,
                        lhsT=scratch[:, 0:128],
                        rhs=scratch,
                        skip_group_check=True,
                    )
                def qk_unit_pre(m, t):
                    ps = psApre.tile([128, 512], F32, tag="pre", name="psqk")
                    qk_mms(ps, m, t)
                    base = HL if m >= 2 else 0
                    hp = 2 * (m % 2)
                    sl = slice(t * 512, (t + 1) * 512)
                    nc.scalar.copy(
                        out=qkT[0:64, base + hp, sl], in_=ps[0:64, :]
                    )
                    nc.vector.tensor_copy(
                        out=qkT[64:128, base + hp + 1, sl], in_=ps[64:128, :]
                    )

                def v_unit_pre(t):
                    ps = psApre.tile([128, 512], F32, tag="pre", name="psv")
                    v_mms(ps[:, 0:VF], t)
                    nc.scalar.copy(
                        out=vaug[:, t, :, 0:DH],
                        in_=ps[:, 0:VF].rearrange("p (h e) -> p h e", h=HL),
                    )

                qk_unit_pre(2, 0)
                qk_unit_pre(0, 0)
                for t in range(5):
                    v_unit_pre(t)

            # PSUM budget (8 banks): psA 1 (filler ring) + psS 4 (score
            # double-buffer) + psB2 3 (AV accumulators + proj).
            with (
                tc.tile_pool(name="psA", bufs=1, space="PSUM") as psA,
                tc.tile_pool(name="psS", bufs=2, space="PSUM") as psS,
                tc.tile_pool(name="psB2", bufs=3, space="PSUM") as psB2,
                tc.tile_pool(name="attnp", bufs=2) as attnp,
                tc.tile_pool(name="outp", bufs=2) as outp,
                tc.tile_pool(name="smalls", bufs=3) as smalls,
            ):
                # ---- leftover phase A as fillers (DVE copies) ----
                def qk_unit(m, t):
                    ps = psA.tile([128, 512], F32, tag="fa", name="psqk")
                    qk_mms(ps, m, t)
                    base = HL if m >= 2 else 0
                    hp = 2 * (m % 2)
                    sl = slice(t * 512, (t + 1) * 512)
                    nc.vector.tensor_copy(
                        out=qkT[0:64, base + hp, sl], in_=ps[0:64, :]
                    )
                    nc.vector.tensor_copy(
                        out=qkT[64:128, base + hp + 1, sl], in_=ps[64:128, :]
                    )

                def v_unit(t):
                    ps = psA.tile([128, 512], F32, tag="fa", name="psv")
                    v_mms(ps[:, 0:VF], t)
                    nc.vector.tensor_copy(
                        out=vaug[:, t, :, 0:DH],
                        in_=ps[:, 0:VF].rearrange("p (h e) -> p h e", h=HL),
                    )

                pending = deque()

                def F(fn, *a):
                    pending.append(lambda: fn(*a))

                # Deadline-ordered fillers (tile-0/pair-0 slot g pops
                # filler #g): v block t before AV(t) at group t+1 (position
                # <= t); k tile t before score kb=4t (position <= 4t);
                # pair-1 q/k before pair 1 starts (position <= 15).
                F(qk_unit, 2, 1); F(v_unit, 5)
                F(qk_unit, 2, 2); F(v_unit, 6)
                F(v_unit, 7)
                F(qk_unit, 2, 3)
                for t in range(8, 16):
                    F(v_unit, t)
                F(qk_unit, 3, 0)
                F(qk_unit, 1, 0)
                # Late fillers: spaced one-apart (None = skip a slot) so
                # each filler's PSUM-ring copy completes before the next
                # filler's matmul, even when the DVE is busy with a
                # normalize chain -- otherwise the in-order PE queue stalls
                # at every pair boundary.
                SK = None
                extras = {
                    0: [(3, 1), SK, (3, 2), SK, (3, 3), SK, (0, 1), SK,
                        (1, 1), SK, (0, 2), SK, (1, 2), SK, (0, 3), SK,
                        (1, 3)],
                }

                # ---- phase B ----
                def slot(kb=KB):
                    if pending:
                        if getattr(pending[0], "is_proj", False) and kb < 10:
                            return
                        pending.popleft()()

                def drain_avp(avp, avc):
                    """Copy both AV accumulators to SBUF, freeing their
                    PSUM banks for the next pair's accumulation."""
                    for i in range(2):
                        avc[i] = smalls.tile(
                            [DH + 1, QT], F32, tag="avc", name="avc"
                        )
                        nc.vector.tensor_copy(out=avc[i], in_=avp[i])

                def normalize(h, avc, outT):
                    j, i = divmod(h, 2)
                    a = avc[i]
                    # shift the denominator row to partition 0 first:
                    # reciprocal_approx_fast mis-executes on HW when its
                    # APs have a nonzero base partition.
                    rd0 = smalls.tile([1, QT], F32, tag="rd0")
                    nc.sync.dma_start(out=rd0, in_=a[DH:DH + 1, :])
                    rr0 = smalls.tile([1, QT], F32, tag="rr0")
                    nc.vector.reciprocal_approx_fast(out=rr0, in_=rd0)
                    rb = smalls.tile([64, QT], F32, tag="rb")
                    nc.gpsimd.partition_broadcast(rb, rr0, channels=64)
                    if i == 0:
                        nc.vector.tensor_mul(
                            outT[0:64, j, :], a[0:DH, :], rb
                        )
                    else:
                        ot = smalls.tile([64, QT], CDT, tag="ot")
                        nc.vector.tensor_mul(ot, a[0:DH, :], rb)
                        nc.sync.dma_start(out=outT[64:128, j, :], in_=ot)

                def proj_unit(outT, n, qb):
                    yps = psB2.tile([128, DO], F32, tag="bank", name="yps")

                    for c in range(NP):
                        nc.tensor.matmul(
                            yps,
                            lhsT=outT[:, c, qb * 128:(qb + 1) * 128],
                            rhs=wo_sb[:, c, :],
                            start=(c == 0),
                            stop=(c == NP - 1),
                            skip_group_check=True,
                        )
                    ysb = smalls.tile([128, DO], F32, tag="ysb", name="ysb")
                    nc.vector.tensor_copy(out=ysb, in_=yps)
                    nc.sync.dma_start(
                        out=y[n * QT + qb * 128:n * QT + (qb + 1) * 128, :],
                        in_=ysb,
                    )

                for n in range(NQT):
                    outT = outp.tile([128, NP, QT], CDT, tag="outT")
                    nsl = slice(n * QT, (n + 1) * QT)
                    for j in range(NP):
                        atp = attnp.tile([128, KB, 2, QT], CDT, tag="atp",
                                         name="atp")
                        avp = [None, None]
                        avcs = [None, None]

                        def av_pair(kb, j=j, atp=atp, avp=avp):
                            for i in range(2):
                                if kb == 0:
                                    avp[i] = psB2.tile(
                                        [DH + 1, QT], F32, tag="bank",
                                        name="avp",
                                    )
                                nc.tensor.matmul(
                                    avp[i],
                                    lhsT=vaug[:, kb, 2 * j + i, :],
                                    rhs=atp[:, kb, i, :],
                                    start=(kb == 0),
                                    stop=(kb == KB - 1),
                                    skip_group_check=True,
                                )

                        def av_tail(avp=avp, avcs=avcs, av_pair=av_pair):
                            """Last AV chunk + drain of both accumulators,
                            popped FIRST in the next pair so its banks free
                            before the next pair's accumulators allocate."""
                            av_pair(KB - 1)
                            drain_avp(avp, avcs)

                        for kb in range(KB):
                            ksl = slice(kb * 128, (kb + 1) * 128)
                            ps = psS.tile([128, 2, 512], F32, tag="sc")
                            for i in range(2):
                                h = 2 * j + i
                                nc.tensor.matmul(
                                    ps[:, i, :],
                                    lhsT=qkT[:, HL + h, ksl],
                                    rhs=qkT[:, h, nsl],
                                    skip_group_check=True,
                                )
                            nc.scalar.activation(
                                out=atp[:, kb, :, :], in_=ps,
                                func=AF.Exp, scale=SCALE,
                            )
                            slot(kb)
                            if kb > 0:
                                av_pair(kb - 1)
                        # pair tail: last AV chunk + accumulator drain +
                        # normalizes pop at the START of the next pair's
                        # slots (ahead of leftover fillers).
                        for mt in extras.pop(2 * n + j, []):
                            if mt is None:
                                pending.append(lambda: None)
                            else:
                                F(qk_unit, mt[0], mt[1])
                        pending.appendleft(
                            lambda h=2 * j, a=avcs, o=outT:
                            normalize(h, a, o)
                        )
                        pending.appendleft(
                            lambda h=2 * j + 1, a=avcs, o=outT:
                            normalize(h, a, o)
                        )
                        pending.appendleft(av_tail)
                    for qb in range(QT // 128):
                        pu = (lambda o=outT, n=n, qb=qb:
                              proj_unit(o, n, qb))
                        pu.is_proj = True
                        pending.append(pu)

                drain_i = 0
                while pending:
                    item = pending.popleft()
                    item()
                    drain_i += 1
                    if drain_i <= 4 and not getattr(item, "is_proj", False):
                        for _ in range(3):
                            wu = psA.tile([128, 512], F32, tag="fa",
                                          name="wu")
                            nc.tensor.matmul(
                                wu,
                                lhsT=scratch[:, 0:128],
                                rhs=scratch,
                                skip_group_check=True,
                            )

    nc.compile()
    return nc


def shard_inputs(x, W_qkv, W_out):
    """Full inputs -> list of 8 per-core input maps."""
    dt = ml_dtypes.bfloat16
    in_maps = []
    for c in range(N_CORES):
        b, g = divmod(c, 2)
        qcols = W_qkv[:, g * 256:(g + 1) * 256]
        kcols = W_qkv[:, INNER + g * 256:INNER + (g + 1) * 256]
        vcols = W_qkv[:, 2 * INNER + g * 256:2 * INNER + (g + 1) * 256]
        # device-tile-major layouts (see dram_tensor decls in build_nc)
        xTl = (x[b].T.reshape(4, 128, 4, 512)        # (c, p, t, u)
               .transpose(1, 2, 0, 3))               # (p, t, c, u)
        wqkl = (np.concatenate([qcols, kcols], axis=1)
                .reshape(4, 128, 4, 128)             # (c, p, m, f)
                .transpose(1, 2, 0, 3))              # (p, m, c, f)
        wvl = vcols.reshape(4, 128, 256).transpose(1, 0, 2)
        wol = W_out[g * 256:(g + 1) * 256, :].reshape(2, 128, 512)            .transpose(1, 0, 2)
        in_maps.append({
            "xT": np.ascontiguousarray(xTl).astype(dt),
            "wqk": np.ascontiguousarray(wqkl).astype(dt),
            "wv": np.ascontiguousarray(wvl).astype(dt),
            "wo": np.ascontiguousarray(wol).astype(dt),
        })
    return in_maps


def gather_output(ys, b_out):
    out = np.empty((B, S, DO), np.float32)
    for b in range(B):
        out[b] = ys[2 * b] + ys[2 * b + 1]
        out[b] += b_out
    return out


_NC_CACHE = {}


def _get_nc():
    if "nc" not in _NC_CACHE:
        _NC_CACHE["nc"] = build_nc()
    return _NC_CACHE["nc"]


def kernel(**inputs):
    x = np.asarray(inputs["x"], np.float32)
    W_qkv = np.asarray(inputs["W_qkv"], np.float32)
    W_out = np.asarray(inputs["W_out"], np.float32)
    b_out = np.asarray(inputs["b_out"], np.float32)

    from concourse.bass_utils import run_bass_kernel_spmd

    nc = _get_nc()
    in_maps = shard_inputs(x, W_qkv, W_out)
    res = run_bass_kernel_spmd(nc, in_maps, core_ids=list(range(N_CORES)))
    ys = [r["y"] for r in res.results]
    return gather_output(ys, b_out)


# revision 33
# speedup vs baseline: 1.7079x; 1.0039x over previous
"""Multi-head attention (B=4, S=2048, D=512, H=8, DH=64) on 8 TRN2 NeuronCores.

Sharding: core c handles batch b = c//2 and head-group g = c%2 (4 of the 8
heads).  Each core computes its QKV projection (columns of W_qkv for its
heads), attention for its 4 heads, and a partial output projection
(rows of W_out for its heads).  The host sums the two partials per batch
and adds the bias.

Per-core device layout (compute in bf16, fp32 PSUM accumulation):
  - host passes x[b] pre-transposed (xT [D, S]); the QKV projection then
    needs no on-device transpose.
  - q/k live PACKED two heads per 128-partition chunk (head 2j in
    partitions 0:64, head 2j+1 in 64:128).  Score matmuls contract over
    just the 64 real rows and use TensorE row tiling (tile_position
    (0,0) / (64,0)) so both heads of a pair compute CONCURRENTLY in the
    two halves of the PE array.
  - exp(scale*s) is fused into the PSUM->SBUF drain on the Scalar
    engine, one ACTIVATE per (pair, k-block) covering both heads
    (128x1024) to amortize the ~350-cycle ACT fixed overhead.  ACT is
    the pacing engine in steady state; everything else hides under it.
  - attn @ V uses V in natural [token, dh] layout augmented with a ones
    column: one PSUM accumulation produces outT_unnorm [dh, q] AND the
    softmax denominator row.
  - normalization: reciprocal_approx_fast of the denominator row,
    DMA-shift to partition 0, gpsimd broadcast across 64 partitions,
    multiply on the Vector engine.
  - output projection: lhsT = outT [128 (2 heads), 128 q] against the
    matching W_out rows, accumulated over head pairs; the result is
    DMA'd to DRAM straight out of PSUM.
  - phase A (QKV projection) runs k-projection-first so the first exp
    lands on ACT ~8us into the kernel; the remaining projection blocks
    are woven into phase B's ACT-bound groups as TensorE filler, using
    a dedicated 1-bank PSUM ring so they never block the score pipeline.
"""

import sys

for _p in ("/opt/trn_rl_repo", "/root/.axon_site/_ro/trn_rl_repo"):
    if _p not in sys.path:
        sys.path.append(_p)

from collections import deque

import ml_dtypes
import numpy as np

import concourse.bass as bass
import concourse.tile as tile
from concourse import bacc, mybir

F32 = mybir.dt.float32
BF16 = mybir.dt.bfloat16
AF = mybir.ActivationFunctionType

# Problem dims (hardcoded per the grading contract).
B, S, D = 4, 2048, 512
H, DH = 8, 64
INNER = H * DH
HL = 4                # heads per core
DO = D                # output dim
QT = 512              # query tile
SCALE = DH ** -0.5

N_CORES = 8


def build_nc():
    KB = S // 128         # k-token blocks
    DC = D // 128         # contraction chunks for the projections
    QKF = 2 * HL * DH     # q+k feature count per core
    MQK = QKF // 128      # qk feature blocks (2 heads each): m0,m1=q m2,m3=k
    VF = HL * DH          # v feature count per core
    NQT = S // QT         # query tiles
    NP = HL // 2          # head pairs

    CDT = BF16

    nc = bacc.Bacc(
        "TRN2", target_bir_lowering=False, debug=False, num_devices=N_CORES
    )
    # All inputs arrive pre-arranged in SBUF-tile-major layouts so each DMA
    # line is 4-16KB contiguous per partition (descriptor-rate bound
    # otherwise).
    xT = nc.dram_tensor("xT", [128, 4, DC, 512], BF16, kind="ExternalInput").ap()
    wqk = nc.dram_tensor(
        "wqk", [128, MQK, DC, 128], BF16, kind="ExternalInput"
    ).ap()
    wv = nc.dram_tensor("wv", [128, DC, VF], BF16, kind="ExternalInput").ap()
    wo = nc.dram_tensor("wo", [128, NP, DO], BF16, kind="ExternalInput").ap()
    y = nc.dram_tensor("y", [S, DO], F32, kind="ExternalOutput").ap()

    with tile.TileContext(nc) as tc:
        with (
            tc.tile_pool(name="weights", bufs=1) as wpool,
            tc.tile_pool(name="big", bufs=1) as big,
        ):
            # ---- loads, ordered by first use: k-projection weights and
            # the first token half lead so the PE starts ~3us in.
            wqk_sb = wpool.tile([128, MQK, DC, 128], CDT)

            def load_wqk(m):
                nc.sync.dma_start(out=wqk_sb[:, m], in_=wqk[:, m])

            xT_sb = big.tile([128, 4, DC, 512], CDT)

            def load_x(t, split=1):
                for h in range(split):
                    c0, c1 = h * DC // split, (h + 1) * DC // split
                    nc.sync.dma_start(
                        out=xT_sb[:, t, c0:c1], in_=xT[:, t, c0:c1]
                    )

            wv_sb = wpool.tile([128, DC, VF], CDT)
            wo_sb = wpool.tile([128, NP, DO], CDT)
            load_wqk(2)
            load_x(0, split=2)
            load_wqk(0)
            nc.sync.dma_start(out=wv_sb, in_=wv)
            load_x(1)
            load_x(2)
            load_x(3)
            load_wqk(3)
            load_wqk(1)
            nc.sync.dma_start(out=wo_sb, in_=wo)

            # qkT zero-padded: chunk h = qT of head h (real rows
            # (h%2)*64..+64, rest 0), chunk HL+h = kT of head h (same
            # padding); score matmuls contract over K=128 (the zero rows
            # contribute nothing -- sub-128 contraction is not supported
            # by this HW stack, it crashes the runtime).
            # Warm-up while the first DMAs land: a 1-element exp pulls the
            # ACT table load (~2.7us) off the critical path, and a dozen
            # junk matmuls ramp the PE out of its low p-state (the clock
            # needs ~3us of continuous work to reach 2.4GHz).
            scratch = big.tile([128, 512], CDT)
            nc.gpsimd.memset(scratch, 0.5)
            scr_exp = big.tile([1, 1], CDT)
            nc.scalar.activation(
                out=scr_exp, in_=scratch[0:1, 0:1], func=AF.Exp
            )

            qkT = big.tile([128, 2 * HL, S], CDT)
            # zero the pad halves on the (otherwise idle) gpsimd engine,
            # chunk by chunk in first-use order so the prefix copies don't
            # wait on one monolithic 14us memset.
            for ch in (4, 5, 0, 1, 6, 7, 2, 3):
                nc.gpsimd.memset(qkT[:, ch, :], 0.0)
            vaug = big.tile([128, KB, HL, DH + 1], CDT)
            ones_col = vaug[:, :, :, DH:DH + 1]
            nc.vector.memset(ones_col, 1.0)

            def qk_mms(ps, m, t):
                for c in range(DC):
                    nc.tensor.matmul(
                        ps,
                        lhsT=wqk_sb[:, m, c, :],
                        rhs=xT_sb[:, t, c, :],
                        start=(c == 0),
                        stop=(c == DC - 1),
                    )

            def v_mms(ps, tb):
                t, sub = divmod(tb, 4)
                for c in range(DC):
                    nc.tensor.matmul(
                        ps,
                        lhsT=xT_sb[:, t, c, sub * 128:(sub + 1) * 128],
                        rhs=wv_sb[:, c, :],
                        start=(c == 0),
                        stop=(c == DC - 1),
                    )

            # Blocking prefix (pipelined through a 4-bank ring that is
            # released before the phase-B pools open): k pair 0 over all
            # tokens, v token-blocks 0..3, q pair 0 tile 0.
            with tc.tile_pool(name="psApre", bufs=4, space="PSUM") as psApre:
                wups = psApre.tile([128, 512], F32, tag="pre", name="wup")
                for _ in range(12):
                    nc.tensor.matmul(
                        wups,
                        lhsT=scratch[:, 0:128],
                        rhs=scratch,
                        skip_group_check=True,
                    )
                def qk_unit_pre(m, t):
                    ps = psApre.tile([128, 512], F32, tag="pre", name="psqk")
                    qk_mms(ps, m, t)
                    base = HL if m >= 2 else 0
                    hp = 2 * (m % 2)
                    sl = slice(t * 512, (t + 1) * 512)
                    nc.scalar.copy(
                        out=qkT[0:64, base + hp, sl], in_=ps[0:64, :]
                    )
                    nc.vector.tensor_copy(
                        out=qkT[64:128, base + hp + 1, sl], in_=ps[64:128, :]
                    )

                def v_unit_pre(t):
                    ps = psApre.tile([128, 512], F32, tag="pre", name="psv")
                    v_mms(ps[:, 0:VF], t)
                    nc.scalar.copy(
                        out=vaug[:, t, :, 0:DH],
                        in_=ps[:, 0:VF].rearrange("p (h e) -> p h e", h=HL),
                    )

                qk_unit_pre(2, 0)
                qk_unit_pre(0, 0)
                for t in range(5):
                    v_unit_pre(t)

            # PSUM budget (8 banks): psA 1 (filler ring) + psS 4 (score
            # double-buffer) + psB2 3 (AV accumulators + proj).
            with (
                tc.tile_pool(name="psA", bufs=1, space="PSUM") as psA,
                tc.tile_pool(name="psS", bufs=2, space="PSUM") as psS,
                tc.tile_pool(name="psB2", bufs=3, space="PSUM") as psB2,
                tc.tile_pool(name="attnp", bufs=2) as attnp,
                tc.tile_pool(name="outp", bufs=2) as outp,
                tc.tile_pool(name="smalls", bufs=3) as smalls,
            ):
                # ---- leftover phase A as fillers (DVE copies) ----
                def qk_unit(m, t):
                    ps = psA.tile([128, 512], F32, tag="fa", name="psqk")
                    qk_mms(ps, m, t)
                    base = HL if m >= 2 else 0
                    hp = 2 * (m % 2)
                    sl = slice(t * 512, (t + 1) * 512)
                    nc.vector.tensor_copy(
                        out=qkT[0:64, base + hp, sl], in_=ps[0:64, :]
                    )
                    nc.vector.tensor_copy(
                        out=qkT[64:128, base + hp + 1, sl], in_=ps[64:128, :]
                    )

                def v_unit(t):
                    ps = psA.tile([128, 512], F32, tag="fa", name="psv")
                    v_mms(ps[:, 0:VF], t)
                    nc.vector.tensor_copy(
                        out=vaug[:, t, :, 0:DH],
                        in_=ps[:, 0:VF].rearrange("p (h e) -> p h e", h=HL),
                    )

                pending = deque()

                def F(fn, *a):
                    pending.append(lambda: fn(*a))

                # Deadline-ordered fillers (tile-0/pair-0 slot g pops
                # filler #g): v block t before AV(t) at group t+1 (position
                # <= t); k tile t before score kb=4t (position <= 4t);
                # pair-1 q/k before pair 1 starts (position <= 15).
                F(qk_unit, 2, 1); F(v_unit, 5)
                F(qk_unit, 2, 2); F(v_unit, 6)
                F(v_unit, 7)
                F(qk_unit, 2, 3)
                for t in range(8, 16):
                    F(v_unit, t)
                F(qk_unit, 3, 0)
                F(qk_unit, 1, 0)
                # Late fillers: spaced one-apart (None = skip a slot) so
                # each filler's PSUM-ring copy completes before the next
                # filler's matmul, even when the DVE is busy with a
                # normalize chain -- otherwise the in-order PE queue stalls
                # at every pair boundary.
                SK = None
                extras = {
                    0: [(3, 1), SK, (3, 2), SK, (3, 3), SK, (0, 1), SK,
                        (1, 1), SK, (0, 2), SK, (1, 2), SK, (0, 3), SK,
                        (1, 3)],
                }

                # ---- phase B ----
                def slot(kb=KB):
                    if pending:
                        if getattr(pending[0], "is_proj", False) and kb < 10:
                            return
                        pending.popleft()()

                def drain_avp(avp, avc, last=False):
                    """Copy both AV accumulators to SBUF, freeing their
                    PSUM banks for the next pair's accumulation.  For the
                    final pair only the denominator rows move (shorter
                    critical chain into the tail); the normalize multiplies
                    then read the accumulator straight from PSUM."""
                    for i in range(2):
                        avc[i] = smalls.tile(
                            [DH + 1, QT], F32, tag="avc", name="avc"
                        )
                        if last:
                            nc.vector.tensor_copy(
                                out=avc[i][DH:DH + 1, :],
                                in_=avp[i][DH:DH + 1, :],
                            )
                        else:
                            nc.vector.tensor_copy(out=avc[i], in_=avp[i])

                def normalize(h, avc, outT, avp=None):
                    j, i = divmod(h, 2)
                    a = avc[i]
                    # shift the denominator row to partition 0 first:
                    # reciprocal_approx_fast mis-executes on HW when its
                    # APs have a nonzero base partition.
                    rd0 = smalls.tile([1, QT], F32, tag="rd0")
                    nc.sync.dma_start(out=rd0, in_=a[DH:DH + 1, :])
                    rr0 = smalls.tile([1, QT], F32, tag="rr0")
                    nc.vector.reciprocal_approx_fast(out=rr0, in_=rd0)
                    rb = smalls.tile([64, QT], F32, tag="rb")
                    nc.gpsimd.partition_broadcast(rb, rr0, channels=64)
                    src_out = avp[i] if avp is not None else a
                    if i == 0:
                        nc.vector.tensor_mul(
                            outT[0:64, j, :], src_out[0:DH, :], rb
                        )
                    else:
                        ot = smalls.tile([64, QT], CDT, tag="ot")
                        nc.vector.tensor_mul(ot, src_out[0:DH, :], rb)
                        nc.sync.dma_start(out=outT[64:128, j, :], in_=ot)

                def proj_unit(outT, n, qb):
                    yps = psB2.tile([128, DO], F32, tag="bank", name="yps")

                    for c in range(NP):
                        nc.tensor.matmul(
                            yps,
                            lhsT=outT[:, c, qb * 128:(qb + 1) * 128],
                            rhs=wo_sb[:, c, :],
                            start=(c == 0),
                            stop=(c == NP - 1),
                            skip_group_check=True,
                        )
                    ysb = smalls.tile([128, DO], F32, tag="ysb", name="ysb")
                    nc.vector.tensor_copy(out=ysb, in_=yps)
                    nc.sync.dma_start(
                        out=y[n * QT + qb * 128:n * QT + (qb + 1) * 128, :],
                        in_=ysb,
                    )

                for n in range(NQT):
                    outT = outp.tile([128, NP, QT], CDT, tag="outT")
                    nsl = slice(n * QT, (n + 1) * QT)
                    for j in range(NP):
                        atp = attnp.tile([128, KB, 2, QT], CDT, tag="atp",
                                         name="atp")
                        avp = [None, None]
                        avcs = [None, None]

                        def av_pair(kb, j=j, atp=atp, avp=avp):
                            for i in range(2):
                                if kb == 0:
                                    avp[i] = psB2.tile(
                                        [DH + 1, QT], F32, tag="bank",
                                        name="avp",
                                    )
                                nc.tensor.matmul(
                                    avp[i],
                                    lhsT=vaug[:, kb, 2 * j + i, :],
                                    rhs=atp[:, kb, i, :],
                                    start=(kb == 0),
                                    stop=(kb == KB - 1),
                                    skip_group_check=True,
                                )

                        last = (n == NQT - 1 and j == NP - 1)

                        def av_tail(avp=avp, avcs=avcs, av_pair=av_pair,
                                    last=last):
                            """Last AV chunk + drain of both accumulators,
                            popped FIRST in the next pair so its banks free
                            before the next pair's accumulators allocate."""
                            av_pair(KB - 1)
                            drain_avp(avp, avcs, last=last)

                        for kb in range(KB):
                            ksl = slice(kb * 128, (kb + 1) * 128)
                            ps = psS.tile([128, 2, 512], F32, tag="sc")
                            for i in range(2):
                                h = 2 * j + i
                                nc.tensor.matmul(
                                    ps[:, i, :],
                                    lhsT=qkT[:, HL + h, ksl],
                                    rhs=qkT[:, h, nsl],
                                    skip_group_check=True,
                                )
                            nc.scalar.activation(
                                out=atp[:, kb, :, :], in_=ps,
                                func=AF.Exp, scale=SCALE,
                            )
                            slot(kb)
                            if kb > 0:
                                av_pair(kb - 1)
                        # pair tail: last AV chunk + accumulator drain +
                        # normalizes pop at the START of the next pair's
                        # slots (ahead of leftover fillers).
                        for mt in extras.pop(2 * n + j, []):
                            if mt is None:
                                pending.append(lambda: None)
                            else:
                                F(qk_unit, mt[0], mt[1])
                        lavp = avp if last else None
                        pending.appendleft(
                            lambda h=2 * j, a=avcs, o=outT, p=lavp:
                            normalize(h, a, o, p)
                        )
                        pending.appendleft(
                            lambda h=2 * j + 1, a=avcs, o=outT, p=lavp:
                            normalize(h, a, o, p)
                        )
                        pending.appendleft(av_tail)
                    for qb in range(QT // 128):
                        pu = (lambda o=outT, n=n, qb=qb:
                              proj_unit(o, n, qb))
                        pu.is_proj = True
                        pending.append(pu)

                drain_i = 0
                while pending:
                    item = pending.popleft()
                    item()
                    drain_i += 1
                    if drain_i <= 4 and not getattr(item, "is_proj", False):
                        for _ in range(3):
                            wu = psA.tile([128, 512], F32, tag="fa",
                                          name="wu")
                            nc.tensor.matmul(
                                wu,
                                lhsT=scratch[:, 0:128],
                                rhs=scratch,
                                skip_group_check=True,
                            )

    nc.compile()
    return nc


def shard_inputs(x, W_qkv, W_out):
    """Full inputs -> list of 8 per-core input maps."""
    dt = ml_dtypes.bfloat16
    in_maps = []
    for c in range(N_CORES):
        b, g = divmod(c, 2)
        qcols = W_qkv[:, g * 256:(g + 1) * 256]
        kcols = W_qkv[:, INNER + g * 256:INNER + (g + 1) * 256]
        vcols = W_qkv[:, 2 * INNER + g * 256:2 * INNER + (g + 1) * 256]
        # device-tile-major layouts (see dram_tensor decls in build_nc)
        xTl = (x[b].T.reshape(4, 128, 4, 512)        # (c, p, t, u)
               .transpose(1, 2, 0, 3))               # (p, t, c, u)
        wqkl = (np.concatenate([qcols, kcols], axis=1)
                .reshape(4, 128, 4, 128)             # (c, p, m, f)
                .transpose(1, 2, 0, 3))              # (p, m, c, f)
        wvl = vcols.reshape(4, 128, 256).transpose(1, 0, 2)
        wol = W_out[g * 256:(g + 1) * 256, :].reshape(2, 128, 512)            .transpose(1, 0, 2)
        in_maps.append({
            "xT": np.ascontiguousarray(xTl).astype(dt),
            "wqk": np.ascontiguousarray(wqkl).astype(dt),
            "wv": np.ascontiguousarray(wvl).astype(dt),
            "wo": np.ascontiguousarray(wol).astype(dt),
        })
    return in_maps


def gather_output(ys, b_out):
    out = np.empty((B, S, DO), np.float32)
    for b in range(B):
        out[b] = ys[2 * b] + ys[2 * b + 1]
        out[b] += b_out
    return out


_NC_CACHE = {}


def _get_nc():
    if "nc" not in _NC_CACHE:
        _NC_CACHE["nc"] = build_nc()
    return _NC_CACHE["nc"]


def kernel(**inputs):
    x = np.asarray(inputs["x"], np.float32)
    W_qkv = np.asarray(inputs["W_qkv"], np.float32)
    W_out = np.asarray(inputs["W_out"], np.float32)
    b_out = np.asarray(inputs["b_out"], np.float32)

    from concourse.bass_utils import run_bass_kernel_spmd

    nc = _get_nc()
    in_maps = shard_inputs(x, W_qkv, W_out)
    res = run_bass_kernel_spmd(nc, in_maps, core_ids=list(range(N_CORES)))
    ys = [r["y"] for r in res.results]
    return gather_output(ys, b_out)


# revision 34
# speedup vs baseline: 1.7148x; 1.0041x over previous
"""Multi-head attention (B=4, S=2048, D=512, H=8, DH=64) on 8 TRN2 NeuronCores.

Sharding: core c handles batch b = c//2 and head-group g = c%2 (4 of the 8
heads).  Each core computes its QKV projection (columns of W_qkv for its
heads), attention for its 4 heads, and a partial output projection
(rows of W_out for its heads).  The host sums the two partials per batch
and adds the bias.

Per-core device layout (compute in bf16, fp32 PSUM accumulation):
  - host passes x[b] pre-transposed (xT [D, S]); the QKV projection then
    needs no on-device transpose.
  - q/k live PACKED two heads per 128-partition chunk (head 2j in
    partitions 0:64, head 2j+1 in 64:128).  Score matmuls contract over
    just the 64 real rows and use TensorE row tiling (tile_position
    (0,0) / (64,0)) so both heads of a pair compute CONCURRENTLY in the
    two halves of the PE array.
  - exp(scale*s) is fused into the PSUM->SBUF drain on the Scalar
    engine, one ACTIVATE per (pair, k-block) covering both heads
    (128x1024) to amortize the ~350-cycle ACT fixed overhead.  ACT is
    the pacing engine in steady state; everything else hides under it.
  - attn @ V uses V in natural [token, dh] layout augmented with a ones
    column: one PSUM accumulation produces outT_unnorm [dh, q] AND the
    softmax denominator row.
  - normalization: reciprocal_approx_fast of the denominator row,
    DMA-shift to partition 0, gpsimd broadcast across 64 partitions,
    multiply on the Vector engine.
  - output projection: lhsT = outT [128 (2 heads), 128 q] against the
    matching W_out rows, accumulated over head pairs; the result is
    DMA'd to DRAM straight out of PSUM.
  - phase A (QKV projection) runs k-projection-first so the first exp
    lands on ACT ~8us into the kernel; the remaining projection blocks
    are woven into phase B's ACT-bound groups as TensorE filler, using
    a dedicated 1-bank PSUM ring so they never block the score pipeline.
"""

import sys

for _p in ("/opt/trn_rl_repo", "/root/.axon_site/_ro/trn_rl_repo"):
    if _p not in sys.path:
        sys.path.append(_p)

from collections import deque

import ml_dtypes
import numpy as np

import concourse.bass as bass
import concourse.tile as tile
from concourse import bacc, mybir

F32 = mybir.dt.float32
BF16 = mybir.dt.bfloat16
AF = mybir.ActivationFunctionType

# Problem dims (hardcoded per the grading contract).
B, S, D = 4, 2048, 512
H, DH = 8, 64
INNER = H * DH
HL = 4                # heads per core
DO = D                # output dim
QT = 512              # query tile
SCALE = DH ** -0.5

N_CORES = 8


def build_nc():
    KB = S // 128         # k-token blocks
    DC = D // 128         # contraction chunks for the projections
    QKF = 2 * HL * DH     # q+k feature count per core
    MQK = QKF // 128      # qk feature blocks (2 heads each): m0,m1=q m2,m3=k
    VF = HL * DH          # v feature count per core
    NQT = S // QT         # query tiles
    NP = HL // 2          # head pairs

    CDT = BF16

    nc = bacc.Bacc(
        "TRN2", target_bir_lowering=False, debug=False, num_devices=N_CORES
    )
    # All inputs arrive pre-arranged in SBUF-tile-major layouts so each DMA
    # line is 4-16KB contiguous per partition (descriptor-rate bound
    # otherwise).
    xT = nc.dram_tensor("xT", [128, 4, DC, 512], BF16, kind="ExternalInput").ap()
    wqk = nc.dram_tensor(
        "wqk", [128, MQK, DC, 128], BF16, kind="ExternalInput"
    ).ap()
    wv = nc.dram_tensor("wv", [128, DC, VF], BF16, kind="ExternalInput").ap()
    wo = nc.dram_tensor("wo", [128, NP, DO], BF16, kind="ExternalInput").ap()
    y = nc.dram_tensor("y", [S, DO], F32, kind="ExternalOutput").ap()

    with tile.TileContext(nc) as tc:
        with (
            tc.tile_pool(name="weights", bufs=1) as wpool,
            tc.tile_pool(name="big", bufs=1) as big,
        ):
            # ---- loads, ordered by first use: k-projection weights and
            # the first token half lead so the PE starts ~3us in.
            wqk_sb = wpool.tile([128, MQK, DC, 128], CDT)

            def load_wqk(m):
                nc.sync.dma_start(out=wqk_sb[:, m], in_=wqk[:, m])

            xT_sb = big.tile([128, 4, DC, 512], CDT)

            def load_x(t, split=1):
                for h in range(split):
                    c0, c1 = h * DC // split, (h + 1) * DC // split
                    nc.sync.dma_start(
                        out=xT_sb[:, t, c0:c1], in_=xT[:, t, c0:c1]
                    )

            wv_sb = wpool.tile([128, DC, VF], CDT)
            wo_sb = wpool.tile([128, NP, DO], CDT)
            load_wqk(2)
            load_x(0, split=2)
            load_wqk(0)
            nc.sync.dma_start(out=wv_sb, in_=wv)
            load_x(1)
            load_x(2)
            load_x(3)
            load_wqk(3)
            load_wqk(1)
            nc.sync.dma_start(out=wo_sb, in_=wo)

            # qkT zero-padded: chunk h = qT of head h (real rows
            # (h%2)*64..+64, rest 0), chunk HL+h = kT of head h (same
            # padding); score matmuls contract over K=128 (the zero rows
            # contribute nothing -- sub-128 contraction is not supported
            # by this HW stack, it crashes the runtime).
            # Warm-up while the first DMAs land: a 1-element exp pulls the
            # ACT table load (~2.7us) off the critical path, and a dozen
            # junk matmuls ramp the PE out of its low p-state (the clock
            # needs ~3us of continuous work to reach 2.4GHz).
            scratch = big.tile([128, 512], CDT)
            nc.gpsimd.memset(scratch, 0.5)
            scr_exp = big.tile([1, 1], CDT)
            nc.scalar.activation(
                out=scr_exp, in_=scratch[0:1, 0:1], func=AF.Exp
            )

            qkT = big.tile([128, 2 * HL, S], CDT)
            # zero the pad halves on the (otherwise idle) gpsimd engine,
            # chunk by chunk in first-use order so the prefix copies don't
            # wait on one monolithic 14us memset.
            for ch in (4, 5, 0, 1, 6, 7, 2, 3):
                nc.gpsimd.memset(qkT[:, ch, :], 0.0)
            vaug = big.tile([128, KB, HL, DH + 1], CDT)
            ones_col = vaug[:, :, :, DH:DH + 1]
            nc.vector.memset(ones_col, 1.0)

            def qk_mms(ps, m, t):
                for c in range(DC):
                    nc.tensor.matmul(
                        ps,
                        lhsT=wqk_sb[:, m, c, :],
                        rhs=xT_sb[:, t, c, :],
                        start=(c == 0),
                        stop=(c == DC - 1),
                    )

            def v_mms(ps, tb):
                t, sub = divmod(tb, 4)
                for c in range(DC):
                    nc.tensor.matmul(
                        ps,
                        lhsT=xT_sb[:, t, c, sub * 128:(sub + 1) * 128],
                        rhs=wv_sb[:, c, :],
                        start=(c == 0),
                        stop=(c == DC - 1),
                    )

            # Blocking prefix (pipelined through a 4-bank ring that is
            # released before the phase-B pools open): k pair 0 over all
            # tokens, v token-blocks 0..3, q pair 0 tile 0.
            with tc.tile_pool(name="psApre", bufs=4, space="PSUM") as psApre:
                wups = psApre.tile([128, 512], F32, tag="pre", name="wup")
                for _ in range(12):
                    nc.tensor.matmul(
                        wups,
                        lhsT=scratch[:, 0:128],
                        rhs=scratch,
                        skip_group_check=True,
                    )
                def qk_unit_pre(m, t):
                    ps = psApre.tile([128, 512], F32, tag="pre", name="psqk")
                    qk_mms(ps, m, t)
                    base = HL if m >= 2 else 0
                    hp = 2 * (m % 2)
                    sl = slice(t * 512, (t + 1) * 512)
                    nc.scalar.copy(
                        out=qkT[0:64, base + hp, sl], in_=ps[0:64, :]
                    )
                    nc.vector.tensor_copy(
                        out=qkT[64:128, base + hp + 1, sl], in_=ps[64:128, :]
                    )

                def v_unit_pre(t):
                    ps = psApre.tile([128, 512], F32, tag="pre", name="psv")
                    v_mms(ps[:, 0:VF], t)
                    nc.scalar.copy(
                        out=vaug[:, t, :, 0:DH],
                        in_=ps[:, 0:VF].rearrange("p (h e) -> p h e", h=HL),
                    )

                qk_unit_pre(2, 0)
                qk_unit_pre(0, 0)
                for t in range(5):
                    v_unit_pre(t)

            # PSUM budget (8 banks): psA 1 (filler ring) + psS 4 (score
            # double-buffer) + psB2 3 (AV accumulators + proj).
            with (
                tc.tile_pool(name="psA", bufs=1, space="PSUM") as psA,
                tc.tile_pool(name="psS", bufs=2, space="PSUM") as psS,
                tc.tile_pool(name="psB2", bufs=3, space="PSUM") as psB2,
                tc.tile_pool(name="attnp", bufs=3) as attnp,
                tc.tile_pool(name="outp", bufs=2) as outp,
                tc.tile_pool(name="smalls", bufs=3) as smalls,
            ):
                # ---- leftover phase A as fillers (DVE copies) ----
                def qk_unit(m, t):
                    ps = psA.tile([128, 512], F32, tag="fa", name="psqk")
                    qk_mms(ps, m, t)
                    base = HL if m >= 2 else 0
                    hp = 2 * (m % 2)
                    sl = slice(t * 512, (t + 1) * 512)
                    nc.vector.tensor_copy(
                        out=qkT[0:64, base + hp, sl], in_=ps[0:64, :]
                    )
                    nc.vector.tensor_copy(
                        out=qkT[64:128, base + hp + 1, sl], in_=ps[64:128, :]
                    )

                def v_unit(t):
                    ps = psA.tile([128, 512], F32, tag="fa", name="psv")
                    v_mms(ps[:, 0:VF], t)
                    nc.vector.tensor_copy(
                        out=vaug[:, t, :, 0:DH],
                        in_=ps[:, 0:VF].rearrange("p (h e) -> p h e", h=HL),
                    )

                pending = deque()

                def F(fn, *a):
                    pending.append(lambda: fn(*a))

                # Deadline-ordered fillers (tile-0/pair-0 slot g pops
                # filler #g): v block t before AV(t) at group t+1 (position
                # <= t); k tile t before score kb=4t (position <= 4t);
                # pair-1 q/k before pair 1 starts (position <= 15).
                F(qk_unit, 2, 1); F(v_unit, 5)
                F(qk_unit, 2, 2); F(v_unit, 6)
                F(v_unit, 7)
                F(qk_unit, 2, 3)
                for t in range(8, 16):
                    F(v_unit, t)
                F(qk_unit, 3, 0)
                F(qk_unit, 1, 0)
                # Late fillers: spaced one-apart (None = skip a slot) so
                # each filler's PSUM-ring copy completes before the next
                # filler's matmul, even when the DVE is busy with a
                # normalize chain -- otherwise the in-order PE queue stalls
                # at every pair boundary.
                SK = None
                extras = {
                    0: [(3, 1), SK, (3, 2), SK, (3, 3), SK, (0, 1), SK,
                        (1, 1), SK, (0, 2), SK, (1, 2), SK, (0, 3), SK,
                        (1, 3)],
                }

                # ---- phase B ----
                def slot(kb=KB):
                    if pending:
                        if getattr(pending[0], "is_proj", False) and kb < 10:
                            return
                        pending.popleft()()

                def drain_avp(avp, avc, last=False):
                    """Copy both AV accumulators to SBUF, freeing their
                    PSUM banks for the next pair's accumulation.  For the
                    final pair only the denominator rows move (shorter
                    critical chain into the tail); the normalize multiplies
                    then read the accumulator straight from PSUM."""
                    for i in range(2):
                        avc[i] = smalls.tile(
                            [DH + 1, QT], F32, tag="avc", name="avc"
                        )
                        if last:
                            nc.vector.tensor_copy(
                                out=avc[i][DH:DH + 1, :],
                                in_=avp[i][DH:DH + 1, :],
                            )
                        else:
                            nc.vector.tensor_copy(out=avc[i], in_=avp[i])

                def normalize(h, avc, outT, avp=None):
                    j, i = divmod(h, 2)
                    a = avc[i]
                    # shift the denominator row to partition 0 first:
                    # reciprocal_approx_fast mis-executes on HW when its
                    # APs have a nonzero base partition.
                    rd0 = smalls.tile([1, QT], F32, tag="rd0")
                    nc.sync.dma_start(out=rd0, in_=a[DH:DH + 1, :])
                    rr0 = smalls.tile([1, QT], F32, tag="rr0")
                    nc.vector.reciprocal_approx_fast(out=rr0, in_=rd0)
                    rb = smalls.tile([64, QT], F32, tag="rb")
                    nc.gpsimd.partition_broadcast(rb, rr0, channels=64)
                    src_out = avp[i] if avp is not None else a
                    if i == 0:
                        nc.vector.tensor_mul(
                            outT[0:64, j, :], src_out[0:DH, :], rb
                        )
                    else:
                        ot = smalls.tile([64, QT], CDT, tag="ot")
                        nc.vector.tensor_mul(ot, src_out[0:DH, :], rb)
                        nc.sync.dma_start(out=outT[64:128, j, :], in_=ot)

                def proj_unit(outT, n, qb):
                    yps = psB2.tile([128, DO], F32, tag="bank", name="yps")

                    for c in range(NP):
                        nc.tensor.matmul(
                            yps,
                            lhsT=outT[:, c, qb * 128:(qb + 1) * 128],
                            rhs=wo_sb[:, c, :],
                            start=(c == 0),
                            stop=(c == NP - 1),
                            skip_group_check=True,
                        )
                    ysb = smalls.tile([128, DO], F32, tag="ysb", name="ysb")
                    nc.vector.tensor_copy(out=ysb, in_=yps)
                    nc.sync.dma_start(
                        out=y[n * QT + qb * 128:n * QT + (qb + 1) * 128, :],
                        in_=ysb,
                    )

                for n in range(NQT):
                    outT = outp.tile([128, NP, QT], CDT, tag="outT")
                    nsl = slice(n * QT, (n + 1) * QT)
                    for j in range(NP):
                        atp = attnp.tile([128, KB, 2, QT], CDT, tag="atp",
                                         name="atp")
                        avp = [None, None]
                        avcs = [None, None]

                        def av_pair(kb, j=j, atp=atp, avp=avp):
                            for i in range(2):
                                if kb == 0:
                                    avp[i] = psB2.tile(
                                        [DH + 1, QT], F32, tag="bank",
                                        name="avp",
                                    )
                                nc.tensor.matmul(
                                    avp[i],
                                    lhsT=vaug[:, kb, 2 * j + i, :],
                                    rhs=atp[:, kb, i, :],
                                    start=(kb == 0),
                                    stop=(kb == KB - 1),
                                    skip_group_check=True,
                                )

                        last = (n == NQT - 1 and j == NP - 1)

                        def av_tail(avp=avp, avcs=avcs, av_pair=av_pair,
                                    last=last):
                            """Last AV chunk + drain of both accumulators,
                            popped FIRST in the next pair so its banks free
                            before the next pair's accumulators allocate."""
                            av_pair(KB - 1)
                            drain_avp(avp, avcs, last=last)

                        for kb in range(KB):
                            ksl = slice(kb * 128, (kb + 1) * 128)
                            ps = psS.tile([128, 2, 512], F32, tag="sc")
                            for i in range(2):
                                h = 2 * j + i
                                nc.tensor.matmul(
                                    ps[:, i, :],
                                    lhsT=qkT[:, HL + h, ksl],
                                    rhs=qkT[:, h, nsl],
                                    skip_group_check=True,
                                )
                            nc.scalar.activation(
                                out=atp[:, kb, :, :], in_=ps,
                                func=AF.Exp, scale=SCALE,
                            )
                            slot(kb)
                            if kb > 0:
                                av_pair(kb - 1)
                        # pair tail: last AV chunk + accumulator drain +
                        # normalizes pop at the START of the next pair's
                        # slots (ahead of leftover fillers).
                        for mt in extras.pop(2 * n + j, []):
                            if mt is None:
                                pending.append(lambda: None)
                            else:
                                F(qk_unit, mt[0], mt[1])
                        lavp = avp if last else None
                        pending.appendleft(
                            lambda h=2 * j, a=avcs, o=outT, p=lavp:
                            normalize(h, a, o, p)
                        )
                        pending.appendleft(
                            lambda h=2 * j + 1, a=avcs, o=outT, p=lavp:
                            normalize(h, a, o, p)
                        )
                        pending.appendleft(av_tail)
                    for qb in range(QT // 128):
                        pu = (lambda o=outT, n=n, qb=qb:
                              proj_unit(o, n, qb))
                        pu.is_proj = True
                        pending.append(pu)

                drain_i = 0
                while pending:
                    item = pending.popleft()
                    item()
                    drain_i += 1
                    if drain_i <= 4 and not getattr(item, "is_proj", False):
                        for _ in range(3):
                            wu = psA.tile([128, 512], F32, tag="fa",
                                          name="wu")
                            nc.tensor.matmul(
                                wu,
                                lhsT=scratch[:, 0:128],
                                rhs=scratch,
                                skip_group_check=True,
                            )

    nc.compile()
    return nc


def shard_inputs(x, W_qkv, W_out):
    """Full inputs -> list of 8 per-core input maps."""
    dt = ml_dtypes.bfloat16
    in_maps = []
    for c in range(N_CORES):
        b, g = divmod(c, 2)
        qcols = W_qkv[:, g * 256:(g + 1) * 256]
        kcols = W_qkv[:, INNER + g * 256:INNER + (g + 1) * 256]
        vcols = W_qkv[:, 2 * INNER + g * 256:2 * INNER + (g + 1) * 256]
        # device-tile-major layouts (see dram_tensor decls in build_nc)
        xTl = (x[b].T.reshape(4, 128, 4, 512)        # (c, p, t, u)
               .transpose(1, 2, 0, 3))               # (p, t, c, u)
        wqkl = (np.concatenate([qcols, kcols], axis=1)
                .reshape(4, 128, 4, 128)             # (c, p, m, f)
                .transpose(1, 2, 0, 3))              # (p, m, c, f)
        wvl = vcols.reshape(4, 128, 256).transpose(1, 0, 2)
        wol = W_out[g * 256:(g + 1) * 256, :].reshape(2, 128, 512)            .transpose(1, 0, 2)
        in_maps.append({
            "xT": np.ascontiguousarray(xTl).astype(dt),
            "wqk": np.ascontiguousarray(wqkl).astype(dt),
            "wv": np.ascontiguousarray(wvl).astype(dt),
            "wo": np.ascontiguousarray(wol).astype(dt),
        })
    return in_maps


def gather_output(ys, b_out):
    out = np.empty((B, S, DO), np.float32)
    for b in range(B):
        out[b] = ys[2 * b] + ys[2 * b + 1]
        out[b] += b_out
    return out


_NC_CACHE = {}


def _get_nc():
    if "nc" not in _NC_CACHE:
        _NC_CACHE["nc"] = build_nc()
    return _NC_CACHE["nc"]


def kernel(**inputs):
    x = np.asarray(inputs["x"], np.float32)
    W_qkv = np.asarray(inputs["W_qkv"], np.float32)
    W_out = np.asarray(inputs["W_out"], np.float32)
    b_out = np.asarray(inputs["b_out"], np.float32)

    from concourse.bass_utils import run_bass_kernel_spmd

    nc = _get_nc()
    in_maps = shard_inputs(x, W_qkv, W_out)
    res = run_bass_kernel_spmd(nc, in_maps, core_ids=list(range(N_CORES)))
    ys = [r["y"] for r in res.results]
    return gather_output(ys, b_out)
